# revision 27
# baseline (speedup 1.0000x reference)
"""Trainium2 Bass kernel for the LIF dense layer (spike output only).

The reference computes
    P_n   = quant8(alpha*P + Q)            (grid 1/128, round-half-even)
    U     = P_n @ quant8(W) + quant8(b) - S
    S_n   = (U > 0.4)
``input_t`` and ``R`` never influence the output (Q_n/U_q are dead,
gamma == 0), so they are never loaded.

All quantized operands are 8-bit integers scaled by 1/128, hence exactly
representable in bf16, and every partial matmul sum is a multiple of 2^-14
below 2^24 -> bf16 matmul with fp32 PSUM accumulation is bit-exact vs the
fp32 reference einsum.  Rounding uses the fp32 magic-number trick
(x + 1.5*2^16) - 1.5*2^16 == round-to-nearest-even onto the 1/128 grid
(two ACT passes; the fp32 SBUF writeback performs the rounding).
The epilogue compares (U - 0.4) > S in one DVE op: U is exact on the
2^-14 grid, so the fp32 subtract of 0.4f never flips the comparison
(error <= 2^-19 vs a >= 2.4e-5 gap to the threshold).

Layout: P and Q are transposed ON THE HOST to i-major, so the quantized
bf16 tile is directly the matmul lhsT (contraction on partitions) -- the
kernel contains no transposes at all.  All host-side staging is packed so
every DMA moves contiguous 8 KiB per partition.  Spikes leave the chip as
uint8 in partition-major layout (4x less write traffic; host upcasts).

Sharding: pure data parallel over the batch dim, 4096 rows per core on 8
NeuronCores; the [512,512] weights / bias are quantized host-side (exact
replication of the reference quantizer) and replicated.
"""

import sys

import numpy as np

sys.path.insert(0, "/opt/trn_rl_repo")

import ml_dtypes

B, IN, OUT = 32768, 512, 512
NCORES = 8
BL = B // NCORES            # rows per core
PART = 128                  # SBUF partitions
KCH = IN // PART            # contraction chunks of 128
G = 4                       # 128-row tiles per pipeline step (1 MiB DMAs)
NTILES = BL // PART
NSUPER = NTILES // G
BS = G * PART               # batch rows per super-tile
# exp(-dt/tau_mem) as computed by XLA fp32 (1 ulp above numpy's expf)
ALPHA = float(np.array(1062312023, np.uint32).view(np.float32))
MAGIC = 98304.0             # 1.5*2^16: fp32 +/- rounds to multiples of 2^-7
QMAX = 127.0 / 128.0
THR = 0.4


def build_nc(enable_asserts=False):
    import concourse.bass as bass
    import concourse.bacc as bacc
    import concourse.mybir as mybir
    from concourse import tile

    OP = mybir.AluOpType
    AF = mybir.ActivationFunctionType
    dt = mybir.dt
    ts = bass.ts

    # Bacc (not plain Bass): its compile() splits multi-sem waits into
    # event semaphores -- TRN2 allows one wait per instruction.
    nc = bacc.Bacc(
        "TRN2",
        target_bir_lowering=False,
        debug=False,
        enable_asserts=enable_asserts,
        num_devices=NCORES,
    )
    # p/q host-transposed to i-major: [p, si, k, b_local]
    p_d = nc.dram_tensor(
        "p", [PART, NSUPER, KCH, BS], dt.float32, kind="ExternalInput"
    ).ap()
    q_d = nc.dram_tensor(
        "q", [PART, NSUPER, KCH, BS], dt.float32, kind="ExternalInput"
    ).ap()
    # s host-packed b-major: [p, si, j, o]
    s_d = nc.dram_tensor(
        "s", [PART, NSUPER, G, OUT], dt.float32, kind="ExternalInput"
    ).ap()
    w_d = nc.dram_tensor("w", [IN, OUT], dt.bfloat16, kind="ExternalInput").ap()
    # spikes leave as uint8, partition-major (host transposes + upcasts);
    # [p, si, j*OUT+o] so each per-super store is contiguous per partition
    o_d = nc.dram_tensor(
        "o", [PART, NSUPER, G * OUT], dt.uint8, kind="ExternalOutput"
    ).ap()

    wv = w_d.rearrange("(k p) o -> p k o", p=PART)

    with tile.TileContext(nc) as tc:
        with (
            tc.tile_pool(name="const", bufs=1) as cpool,
            tc.tile_pool(name="io", bufs=4) as iop,
            tc.tile_pool(name="work", bufs=2) as wkp,
            tc.tile_pool(name="out", bufs=3) as outp,
            tc.tile_pool(name="psu", bufs=4, space="PSUM") as psp_u,
        ):
            # weights ride the ACT HWDGE ring, AFTER the first q load
            # (their first consumer runs ~15us in; q0 gates everything)
            w_sb = cpool.tile([PART, KCH, OUT], dt.bfloat16)
            # zero K=1,N=1 matmul operands: each PSUM group ends with this
            # no-op so the group's completion semaphore fires only after the
            # systolic pipeline has drained the real matmuls' columns into
            # PSUM (the DVE read below races the drain otherwise)
            z_l = cpool.tile([1, PART], dt.bfloat16)
            nc.vector.memset(z_l[:], 0.0)
            z_r = cpool.tile([1, 1], dt.bfloat16)
            nc.vector.memset(z_r[:], 0.0)

            for si in range(NSUPER):
                p_t = iop.tile([PART, KCH, BS], dt.float32, tag="p")
                q_t = iop.tile([PART, KCH, BS], dt.float32, tag="q")
                s_t = iop.tile([PART, G, OUT], dt.float32, tag="s")
                # loads split across both HWDGE rings: a single FIFO ring
                # leaves ~1.5us completion bubbles between 1 MiB transfers
                nc.sync.dma_start(out=p_t[:], in_=p_d[:, si])
                nc.scalar.dma_start(out=q_t[:], in_=q_d[:, si])
                nc.sync.dma_start(out=s_t[:], in_=s_d[:, si])
                if si == 0:
                    nc.scalar.dma_start(out=w_sb[:], in_=wv[:])

                # x = alpha*P + Q (one DVE pass; each ALU slice rounds fp32)
                x_t = wkp.tile([PART, KCH, BS], dt.float32, tag="x")
                nc.vector.scalar_tensor_tensor(
                    out=x_t[:], in0=p_t[:], scalar=ALPHA, in1=q_t[:],
                    op0=OP.mult, op1=OP.add,
                )
                # round-half-even onto the 1/128 grid.  Middle super-tiles
                # use the ACT engine (2 passes) for throughput; the first
                # and last use one chained DVE op so the pipeline-critical
                # chain stays in-order on a single engine.
                if 0 < si < NSUPER - 1:
                    nc.scalar.activation(x_t[:], x_t[:], AF.Copy, bias=MAGIC)
                    nc.scalar.activation(x_t[:], x_t[:], AF.Copy, bias=-MAGIC)
                else:
                    nc.vector.tensor_scalar(
                        out=x_t[:], in0=x_t[:], scalar1=MAGIC, scalar2=MAGIC,
                        op0=OP.add, op1=OP.subtract,
                    )
                # saturate to +/-127/128 and narrow to bf16 (exact);
                # this IS the matmul lhsT: [i mod 128, k, b_local]
                q8_t = wkp.tile([PART, KCH, BS], dt.bfloat16, tag="q8")
                nc.vector.tensor_scalar(
                    out=q8_t[:], in0=x_t[:], scalar1=QMAX, scalar2=-QMAX,
                    op0=OP.min, op1=OP.max,
                )
                sp_t = outp.tile([PART, G, OUT], dt.uint8, tag="sp")
                for j in range(G):
                    up = psp_u.tile([PART, OUT], dt.float32, tag="up")
                    for k in range(KCH):
                        nc.tensor.matmul(
                            up[:],
                            lhsT=q8_t[:, k, ts(j, PART)],
                            rhs=w_sb[:, k, :],
                            start=(k == 0),
                            stop=False,
                        )
                    # pipeline-drain no-op (adds 0.0 to one element)
                    nc.tensor.matmul(
                        up[:, 0:1], lhsT=z_l[:], rhs=z_r[:],
                        start=False, stop=True,
                    )
                    # s was pre-biased on the host to S + (0.4 - bq), so
                    # spike == (E + bq - S > 0.4) == (E > s'): one DVE op,
                    # narrowed to uint8
                    nc.vector.scalar_tensor_tensor(
                        out=sp_t[:, j, :], in0=up[:], scalar=0.0,
                        in1=s_t[:, j, :], op0=OP.bypass, op1=OP.is_gt,
                    )
                # store on the SWDGE ring: keeps both HWDGE rings for loads
                nc.gpsimd.dma_start(out=o_d[:, si, :], in_=sp_t[:])
    nc.finalize()  # Bacc.compile(): splits multi-sem waits (TRN2 1-wait rule)
    return nc


def _quant_host(x):
    """Exact replica of the reference quant_ste forward pass (fp32)."""
    x = np.asarray(x, np.float32)
    d = np.float32(1.0) / np.float32(128.0)
    y = np.clip(x, np.float32(-1.0) + d, np.float32(1.0) - d)
    y = y * np.float32(128.0)
    y = np.round(y)  # round-half-even, same as jnp.round
    return (y / np.float32(128.0)).astype(np.float32)


def _imajor(a):
    """[BL, IN] -> [PART, NSUPER, KCH, BS] with a[b, i] at [i%128, b//BS, i//128, b%BS]."""
    return np.ascontiguousarray(
        a.reshape(NSUPER, BS, KCH, PART).transpose(3, 0, 2, 1)
    )


def _bmajor(a):
    """[BL, OUT] -> [PART, NSUPER, G, OUT] with a[b, o] at [b%128, b//BS, (b//128)%G, o]."""
    return np.ascontiguousarray(
        a.reshape(NSUPER, G, PART, OUT).transpose(2, 0, 1, 3)
    )


_cache = {}


def kernel(**inputs):
    from concourse.bass_utils import run_bass_kernel_spmd

    P = np.asarray(inputs["P"], np.float32)
    Q = np.asarray(inputs["Q"], np.float32)
    S = np.asarray(inputs["S"], np.float32)
    W = np.asarray(inputs["weights"], np.float32)
    bias = np.asarray(inputs["bias"], np.float32)

    wq = _quant_host(W).astype(ml_dtypes.bfloat16)
    # pre-bias S on the host: spike == (E + bq - S > 0.4) == (E > S + cf);
    # fp32 host arithmetic matches what the chip would compute
    cf = (np.float32(THR) - _quant_host(bias)).astype(np.float32)
    S2 = (S + cf[None, :]).astype(np.float32)

    if "nc" not in _cache:
        _cache["nc"] = build_nc()
    nc = _cache["nc"]

    in_maps = []
    for c in range(NCORES):
        sl = slice(c * BL, (c + 1) * BL)
        in_maps.append(
            {
                "p": _imajor(P[sl]),
                "q": _imajor(Q[sl]),
                "s": _bmajor(S2[sl]),
                "w": wq,
            }
        )
    res = run_bass_kernel_spmd(nc, in_maps, list(range(NCORES)))
    _cache["last"] = res  # exec_time_ns etc. when tracing is enabled
    # device layout: o[p, si, j*OUT + o'] holds row 512*si + 128*j + p
    out = np.concatenate(
        [
            res.results[c]["o"]
            .reshape(PART, NSUPER, G, OUT)
            .transpose(1, 2, 0, 3)
            .reshape(BL, OUT)
            for c in range(NCORES)
        ],
        axis=0,
    )
    return np.ascontiguousarray(out.astype(np.float32))


# revision 29
# speedup vs baseline: 1.0100x; 1.0100x over previous
"""Trainium2 Bass kernel for the LIF dense layer (spike output only).

The reference computes
    P_n   = quant8(alpha*P + Q)            (grid 1/128, round-half-even)
    U     = P_n @ quant8(W) + quant8(b) - S
    S_n   = (U > 0.4)
``input_t`` and ``R`` never influence the output (Q_n/U_q are dead,
gamma == 0), so they are never loaded.

All quantized operands are 8-bit integers scaled by 1/128, hence exactly
representable in bf16, and every partial matmul sum is a multiple of 2^-14
below 2^24 -> bf16 matmul with fp32 PSUM accumulation is bit-exact vs the
fp32 reference einsum.  Rounding uses the fp32 magic-number trick
(x + 1.5*2^16) - 1.5*2^16 == round-to-nearest-even onto the 1/128 grid
(two ACT passes; the fp32 SBUF writeback performs the rounding).
The epilogue compares (U - 0.4) > S in one DVE op: U is exact on the
2^-14 grid, so the fp32 subtract of 0.4f never flips the comparison
(error <= 2^-19 vs a >= 2.4e-5 gap to the threshold).

Layout: P and Q are transposed ON THE HOST to i-major, so the quantized
bf16 tile is directly the matmul lhsT (contraction on partitions) -- the
kernel contains no transposes at all.  All host-side staging is packed so
every DMA moves contiguous 8 KiB per partition.  Spikes leave the chip as
uint8 in partition-major layout (4x less write traffic; host upcasts).

Sharding: pure data parallel over the batch dim, 4096 rows per core on 8
NeuronCores; the [512,512] weights / bias are quantized host-side (exact
replication of the reference quantizer) and replicated.
"""

import sys

import numpy as np

sys.path.insert(0, "/opt/trn_rl_repo")

import ml_dtypes

B, IN, OUT = 32768, 512, 512
NCORES = 8
BL = B // NCORES            # rows per core
PART = 128                  # SBUF partitions
KCH = IN // PART            # contraction chunks of 128
G = 4                       # 128-row tiles per pipeline step (1 MiB DMAs)
NTILES = BL // PART
NSUPER = NTILES // G
BS = G * PART               # batch rows per super-tile
# exp(-dt/tau_mem) as computed by XLA fp32 (1 ulp above numpy's expf)
ALPHA = float(np.array(1062312023, np.uint32).view(np.float32))
MAGIC = 98304.0             # 1.5*2^16: fp32 +/- rounds to multiples of 2^-7
QMAX = 127.0 / 128.0
THR = 0.4


def build_nc(enable_asserts=False):
    import concourse.bass as bass
    import concourse.bacc as bacc
    import concourse.mybir as mybir
    from concourse import tile

    OP = mybir.AluOpType
    AF = mybir.ActivationFunctionType
    dt = mybir.dt
    ts = bass.ts

    # Bacc (not plain Bass): its compile() splits multi-sem waits into
    # event semaphores -- TRN2 allows one wait per instruction.
    nc = bacc.Bacc(
        "TRN2",
        target_bir_lowering=False,
        debug=False,
        enable_asserts=enable_asserts,
        num_devices=NCORES,
    )
    # p/q host-transposed to i-major: [p, si, k, b_local]
    p_d = nc.dram_tensor(
        "p", [PART, NSUPER, KCH, BS], dt.float32, kind="ExternalInput"
    ).ap()
    q_d = nc.dram_tensor(
        "q", [PART, NSUPER, KCH, BS], dt.float32, kind="ExternalInput"
    ).ap()
    # s host-packed b-major: [p, si, j, o]
    s_d = nc.dram_tensor(
        "s", [PART, NSUPER, G, OUT], dt.float32, kind="ExternalInput"
    ).ap()
    w_d = nc.dram_tensor("w", [IN, OUT], dt.bfloat16, kind="ExternalInput").ap()
    # spikes leave as uint8, partition-major (host transposes + upcasts);
    # [p, si, j*OUT+o] so each per-super store is contiguous per partition
    o_d = nc.dram_tensor(
        "o", [PART, NSUPER, G * OUT], dt.uint8, kind="ExternalOutput"
    ).ap()

    wv = w_d.rearrange("(k p) o -> p k o", p=PART)

    with tile.TileContext(nc) as tc:
        with (
            tc.tile_pool(name="const", bufs=1) as cpool,
            tc.tile_pool(name="io", bufs=4) as iop,
            tc.tile_pool(name="work", bufs=2) as wkp,
            tc.tile_pool(name="out", bufs=3) as outp,
            tc.tile_pool(name="psu", bufs=4, space="PSUM") as psp_u,
        ):
            # weights ride the ACT HWDGE ring, AFTER the first q load
            # (their first consumer runs ~15us in; q0 gates everything)
            w_sb = cpool.tile([PART, KCH, OUT], dt.bfloat16)
            # zero K=1,N=1 matmul operands: each PSUM group ends with this
            # no-op so the group's completion semaphore fires only after the
            # systolic pipeline has drained the real matmuls' columns into
            # PSUM (the DVE read below races the drain otherwise)
            z_l = cpool.tile([1, PART], dt.bfloat16)
            nc.vector.memset(z_l[:], 0.0)
            z_r = cpool.tile([1, 1], dt.bfloat16)
            nc.vector.memset(z_r[:], 0.0)

            for si in range(NSUPER):
                # first/last super-tiles process b in halves: the j-tile
                # matmuls only need their own b-slice of q8, so halving
                # pulls the first matmul ~6us earlier and drains the tail
                # in half-super chunks
                edge = si in (0, NSUPER - 1)
                chunks = (
                    [(slice(0, BS // 2), range(0, G // 2)),
                     (slice(BS // 2, BS), range(G // 2, G))]
                    if edge
                    else [(slice(0, BS), range(G))]
                )
                p_t = iop.tile([PART, KCH, BS], dt.float32, tag="p")
                q_t = iop.tile([PART, KCH, BS], dt.float32, tag="q")
                s_t = iop.tile([PART, G, OUT], dt.float32, tag="s")
                x_t = wkp.tile([PART, KCH, BS], dt.float32, tag="x")
                q8_t = wkp.tile([PART, KCH, BS], dt.bfloat16, tag="q8")
                sp_t = outp.tile([PART, G, OUT], dt.uint8, tag="sp")

                for ci, (bsl, jrange) in enumerate(chunks):
                    # loads split across both HWDGE rings: a single FIFO
                    # ring leaves ~1.5us bubbles between 1 MiB transfers
                    nc.sync.dma_start(
                        out=p_t[:, :, bsl], in_=p_d[:, si, :, bsl]
                    )
                    nc.scalar.dma_start(
                        out=q_t[:, :, bsl], in_=q_d[:, si, :, bsl]
                    )
                    jlist = list(jrange)
                    ssl = slice(jlist[0], jlist[-1] + 1)
                    nc.sync.dma_start(
                        out=s_t[:, ssl, :], in_=s_d[:, si, ssl, :]
                    )
                    if si == 0 and ci == 0:
                        nc.scalar.dma_start(out=w_sb[:], in_=wv[:])

                    # x = alpha*P + Q (one DVE pass; fp32 per ALU slice)
                    nc.vector.scalar_tensor_tensor(
                        out=x_t[:, :, bsl], in0=p_t[:, :, bsl],
                        scalar=ALPHA, in1=q_t[:, :, bsl],
                        op0=OP.mult, op1=OP.add,
                    )
                    # round-half-even onto the 1/128 grid.  Middle supers
                    # use the ACT engine (2 passes) for throughput; the
                    # edge supers use one chained DVE op so the critical
                    # chain stays in-order on a single engine.
                    if not edge:
                        nc.scalar.activation(
                            x_t[:, :, bsl], x_t[:, :, bsl], AF.Copy,
                            bias=MAGIC,
                        )
                        nc.scalar.activation(
                            x_t[:, :, bsl], x_t[:, :, bsl], AF.Copy,
                            bias=-MAGIC,
                        )
                    else:
                        nc.vector.tensor_scalar(
                            out=x_t[:, :, bsl], in0=x_t[:, :, bsl],
                            scalar1=MAGIC, scalar2=MAGIC,
                            op0=OP.add, op1=OP.subtract,
                        )
                    # saturate to +/-127/128 and narrow to bf16 (exact);
                    # this IS the matmul lhsT: [i mod 128, k, b_local]
                    nc.vector.tensor_scalar(
                        out=q8_t[:, :, bsl], in0=x_t[:, :, bsl],
                        scalar1=QMAX, scalar2=-QMAX,
                        op0=OP.min, op1=OP.max,
                    )
                    for j in jlist:
                        up = psp_u.tile([PART, OUT], dt.float32, tag="up")
                        for k in range(KCH):
                            nc.tensor.matmul(
                                up[:],
                                lhsT=q8_t[:, k, ts(j, PART)],
                                rhs=w_sb[:, k, :],
                                start=(k == 0),
                                stop=False,
                            )
                        # pipeline-drain no-op (adds 0.0 to one element)
                        nc.tensor.matmul(
                            up[:, 0:1], lhsT=z_l[:], rhs=z_r[:],
                            start=False, stop=True,
                        )
                        # s was pre-biased on the host to S + (0.4 - bq):
                        # spike == (E + bq - S > 0.4) == (E > s')
                        nc.vector.scalar_tensor_tensor(
                            out=sp_t[:, j, :], in0=up[:], scalar=0.0,
                            in1=s_t[:, j, :], op0=OP.bypass, op1=OP.is_gt,
                        )
                    if edge:
                        osl = slice(
                            jlist[0] * OUT, (jlist[-1] + 1) * OUT
                        )
                        nc.gpsimd.dma_start(
                            out=o_d[:, si, osl], in_=sp_t[:, ssl, :]
                        )
                if not edge:
                    # store on the SWDGE ring: keeps both HWDGE rings free
                    nc.gpsimd.dma_start(out=o_d[:, si, :], in_=sp_t[:])
    nc.finalize()  # Bacc.compile(): splits multi-sem waits (TRN2 1-wait rule)
    return nc


def _quant_host(x):
    """Exact replica of the reference quant_ste forward pass (fp32)."""
    x = np.asarray(x, np.float32)
    d = np.float32(1.0) / np.float32(128.0)
    y = np.clip(x, np.float32(-1.0) + d, np.float32(1.0) - d)
    y = y * np.float32(128.0)
    y = np.round(y)  # round-half-even, same as jnp.round
    return (y / np.float32(128.0)).astype(np.float32)


def _imajor(a):
    """[BL, IN] -> [PART, NSUPER, KCH, BS] with a[b, i] at [i%128, b//BS, i//128, b%BS]."""
    return np.ascontiguousarray(
        a.reshape(NSUPER, BS, KCH, PART).transpose(3, 0, 2, 1)
    )


def _bmajor(a):
    """[BL, OUT] -> [PART, NSUPER, G, OUT] with a[b, o] at [b%128, b//BS, (b//128)%G, o]."""
    return np.ascontiguousarray(
        a.reshape(NSUPER, G, PART, OUT).transpose(2, 0, 1, 3)
    )


_cache = {}


def kernel(**inputs):
    from concourse.bass_utils import run_bass_kernel_spmd

    P = np.asarray(inputs["P"], np.float32)
    Q = np.asarray(inputs["Q"], np.float32)
    S = np.asarray(inputs["S"], np.float32)
    W = np.asarray(inputs["weights"], np.float32)
    bias = np.asarray(inputs["bias"], np.float32)

    wq = _quant_host(W).astype(ml_dtypes.bfloat16)
    # pre-bias S on the host: spike == (E + bq - S > 0.4) == (E > S + cf);
    # fp32 host arithmetic matches what the chip would compute
    cf = (np.float32(THR) - _quant_host(bias)).astype(np.float32)
    S2 = (S + cf[None, :]).astype(np.float32)

    if "nc" not in _cache:
        _cache["nc"] = build_nc()
    nc = _cache["nc"]

    in_maps = []
    for c in range(NCORES):
        sl = slice(c * BL, (c + 1) * BL)
        in_maps.append(
            {
                "p": _imajor(P[sl]),
                "q": _imajor(Q[sl]),
                "s": _bmajor(S2[sl]),
                "w": wq,
            }
        )
    res = run_bass_kernel_spmd(nc, in_maps, list(range(NCORES)))
    _cache["last"] = res  # exec_time_ns etc. when tracing is enabled
    # device layout: o[p, si, j*OUT + o'] holds row 512*si + 128*j + p
    out = np.concatenate(
        [
            res.results[c]["o"]
            .reshape(PART, NSUPER, G, OUT)
            .transpose(1, 2, 0, 3)
            .reshape(BL, OUT)
            for c in range(NCORES)
        ],
        axis=0,
    )
    return np.ascontiguousarray(out.astype(np.float32))


# revision 32
# speedup vs baseline: 1.0389x; 1.0286x over previous
"""Trainium2 Bass kernel for the LIF dense layer (spike output only).

The reference computes
    P_n   = quant8(alpha*P + Q)            (grid 1/128, round-half-even)
    U     = P_n @ quant8(W) + quant8(b) - S
    S_n   = (U > 0.4)
``input_t`` and ``R`` never influence the output (Q_n/U_q are dead,
gamma == 0), so they are never loaded.

All quantized operands are 8-bit integers scaled by 1/128, hence exactly
representable in bf16, and every partial matmul sum is a multiple of 2^-14
below 2^24 -> bf16 matmul with fp32 PSUM accumulation is bit-exact vs the
fp32 reference einsum.  Rounding uses the fp32 magic-number trick
(x + 1.5*2^16) - 1.5*2^16 == round-to-nearest-even onto the 1/128 grid
(two ACT passes; the fp32 SBUF writeback performs the rounding).
The epilogue compares (U - 0.4) > S in one DVE op: U is exact on the
2^-14 grid, so the fp32 subtract of 0.4f never flips the comparison
(error <= 2^-19 vs a >= 2.4e-5 gap to the threshold).

Layout: P and Q are transposed ON THE HOST to i-major, so the quantized
bf16 tile is directly the matmul lhsT (contraction on partitions) -- the
kernel contains no transposes at all.  All host-side staging is packed so
every DMA moves contiguous 8 KiB per partition.  Spikes leave the chip as
uint8 in partition-major layout (4x less write traffic; host upcasts).

Sharding: pure data parallel over the batch dim, 4096 rows per core on 8
NeuronCores; the [512,512] weights / bias are quantized host-side (exact
replication of the reference quantizer) and replicated.
"""

import sys

import numpy as np

sys.path.insert(0, "/opt/trn_rl_repo")

import ml_dtypes

B, IN, OUT = 32768, 512, 512
NCORES = 8
BL = B // NCORES            # rows per core
PART = 128                  # SBUF partitions
KCH = IN // PART            # contraction chunks of 128
G = 4                       # 128-row tiles per pipeline step (1 MiB DMAs)
NTILES = BL // PART
NSUPER = NTILES // G
BS = G * PART               # batch rows per super-tile
# exp(-dt/tau_mem) as computed by XLA fp32 (1 ulp above numpy's expf)
ALPHA = float(np.array(1062312023, np.uint32).view(np.float32))
MAGIC = 98304.0             # 1.5*2^16: fp32 +/- rounds to multiples of 2^-7
QMAX = 127.0 / 128.0
THR = 0.4


def build_nc(enable_asserts=False):
    import concourse.bass as bass
    import concourse.bacc as bacc
    import concourse.mybir as mybir
    from concourse import tile

    OP = mybir.AluOpType
    AF = mybir.ActivationFunctionType
    dt = mybir.dt
    ts = bass.ts

    # Bacc (not plain Bass): its compile() splits multi-sem waits into
    # event semaphores -- TRN2 allows one wait per instruction.
    nc = bacc.Bacc(
        "TRN2",
        target_bir_lowering=False,
        debug=False,
        enable_asserts=enable_asserts,
        num_devices=NCORES,
    )
    # p/q host-transposed to i-major: [p, si, k, b_local]
    p_d = nc.dram_tensor(
        "p", [PART, NSUPER, KCH, BS], dt.float32, kind="ExternalInput"
    ).ap()
    q_d = nc.dram_tensor(
        "q", [PART, NSUPER, KCH, BS], dt.float32, kind="ExternalInput"
    ).ap()
    # s host-packed b-major: [p, si, j, o]
    s_d = nc.dram_tensor(
        "s", [PART, NSUPER, G, OUT], dt.float32, kind="ExternalInput"
    ).ap()
    w_d = nc.dram_tensor("w", [IN, OUT], dt.bfloat16, kind="ExternalInput").ap()
    # spikes leave as uint8, partition-major (host transposes + upcasts);
    # [p, si, j*OUT+o] so each per-super store is contiguous per partition
    o_d = nc.dram_tensor(
        "o", [PART, NSUPER, G * OUT], dt.uint8, kind="ExternalOutput"
    ).ap()

    wv = w_d.rearrange("(k p) o -> p k o", p=PART)

    with tile.TileContext(nc) as tc:
        with (
            tc.tile_pool(name="const", bufs=1) as cpool,
            tc.tile_pool(name="io", bufs=4) as iop,
            tc.tile_pool(name="work", bufs=2) as wkp,
            tc.tile_pool(name="out", bufs=3) as outp,
            tc.tile_pool(name="psu", bufs=4, space="PSUM") as psp_u,
            tc.tile_pool(name="psw", bufs=1, space="PSUM") as psp_w,
        ):
            # weights ride the ACT HWDGE ring, AFTER the first q load
            # (their first consumer runs ~15us in; q0 gates everything)
            w_sb = cpool.tile([PART, KCH, OUT], dt.bfloat16)
            # zero K=1,N=1 matmul operands: each PSUM group ends with this
            # no-op so the group's completion semaphore fires only after the
            # systolic pipeline has drained the real matmuls' columns into
            # PSUM (the DVE read below races the drain otherwise)
            z_l = cpool.tile([1, PART], dt.bfloat16)
            nc.vector.memset(z_l[:], 0.0)
            z_r = cpool.tile([1, 1], dt.bfloat16)
            nc.vector.memset(z_r[:], 0.0)
            z_row = cpool.tile([1, OUT], dt.bfloat16)
            nc.vector.memset(z_row[:], 0.0)

            # dummy matmuls fill the PE's initial DMA-wait window (~7-15us)
            # so the HAM clock gate is already at 8/8 (2.4 GHz) when the
            # first real matmul issues (HAM needs ~3.4us of sustained PE
            # activity and re-throttles after ~3.4us idle)
            warm = psp_w.tile([PART, OUT], dt.float32)
            for _ in range(14):
                nc.tensor.matmul(
                    warm[:], lhsT=z_l[:], rhs=z_row[:], start=True, stop=True
                )

            for si in range(NSUPER):
                # first/last super-tiles process b in halves: the j-tile
                # matmuls only need their own b-slice of q8, so halving
                # pulls the first matmul ~6us earlier and drains the tail
                # in half-super chunks
                edge = si in (0, NSUPER - 1)
                chunks = (
                    [(slice(0, BS // 2), range(0, G // 2)),
                     (slice(BS // 2, BS), range(G // 2, G))]
                    if edge
                    else [(slice(0, BS), range(G))]
                )
                p_t = iop.tile([PART, KCH, BS], dt.float32, tag="p")
                q_t = iop.tile([PART, KCH, BS], dt.float32, tag="q")
                s_t = iop.tile([PART, G, OUT], dt.float32, tag="s")
                x_t = wkp.tile([PART, KCH, BS], dt.float32, tag="x")
                q8_t = wkp.tile([PART, KCH, BS], dt.bfloat16, tag="q8")
                sp_t = outp.tile([PART, G, OUT], dt.uint8, tag="sp")

                for ci, (bsl, jrange) in enumerate(chunks):
                    # loads split across both HWDGE rings: a single FIFO
                    # ring leaves ~1.5us bubbles between 1 MiB transfers
                    nc.sync.dma_start(
                        out=p_t[:, :, bsl], in_=p_d[:, si, :, bsl]
                    )
                    nc.scalar.dma_start(
                        out=q_t[:, :, bsl], in_=q_d[:, si, :, bsl]
                    )
                    jlist = list(jrange)
                    ssl = slice(jlist[0], jlist[-1] + 1)
                    nc.sync.dma_start(
                        out=s_t[:, ssl, :], in_=s_d[:, si, ssl, :]
                    )
                    if si == 0 and ci == 0:
                        nc.scalar.dma_start(out=w_sb[:], in_=wv[:])

                    # x = alpha*P + Q (one DVE pass; fp32 per ALU slice)
                    nc.vector.scalar_tensor_tensor(
                        out=x_t[:, :, bsl], in0=p_t[:, :, bsl],
                        scalar=ALPHA, in1=q_t[:, :, bsl],
                        op0=OP.mult, op1=OP.add,
                    )
                    # round-half-even onto the 1/128 grid.  Middle supers
                    # use the ACT engine (2 passes) for throughput; the
                    # edge supers use one chained DVE op so the critical
                    # chain stays in-order on a single engine.
                    if not edge:
                        nc.scalar.activation(
                            x_t[:, :, bsl], x_t[:, :, bsl], AF.Copy,
                            bias=MAGIC,
                        )
                        nc.scalar.activation(
                            x_t[:, :, bsl], x_t[:, :, bsl], AF.Copy,
                            bias=-MAGIC,
                        )
                    else:
                        nc.vector.tensor_scalar(
                            out=x_t[:, :, bsl], in0=x_t[:, :, bsl],
                            scalar1=MAGIC, scalar2=MAGIC,
                            op0=OP.add, op1=OP.subtract,
                        )
                    # saturate to +/-127/128 and narrow to bf16 (exact);
                    # this IS the matmul lhsT: [i mod 128, k, b_local]
                    nc.vector.tensor_scalar(
                        out=q8_t[:, :, bsl], in0=x_t[:, :, bsl],
                        scalar1=QMAX, scalar2=-QMAX,
                        op0=OP.min, op1=OP.max,
                    )
                    for j in jlist:
                        up = psp_u.tile([PART, OUT], dt.float32, tag="up")
                        for k in range(KCH):
                            nc.tensor.matmul(
                                up[:],
                                lhsT=q8_t[:, k, ts(j, PART)],
                                rhs=w_sb[:, k, :],
                                start=(k == 0),
                                stop=False,
                            )
                        # pipeline-drain no-op (adds 0.0 to one element)
                        nc.tensor.matmul(
                            up[:, 0:1], lhsT=z_l[:], rhs=z_r[:],
                            start=False, stop=True,
                        )
                        # s was pre-biased on the host to S + (0.4 - bq):
                        # spike == (E + bq - S > 0.4) == (E > s')
                        nc.vector.scalar_tensor_tensor(
                            out=sp_t[:, j, :], in0=up[:], scalar=0.0,
                            in1=s_t[:, j, :], op0=OP.bypass, op1=OP.is_gt,
                        )
                    if edge:
                        osl = slice(
                            jlist[0] * OUT, (jlist[-1] + 1) * OUT
                        )
                        nc.gpsimd.dma_start(
                            out=o_d[:, si, osl], in_=sp_t[:, ssl, :]
                        )
                if not edge:
                    # store on the SWDGE ring: keeps both HWDGE rings free
                    nc.gpsimd.dma_start(out=o_d[:, si, :], in_=sp_t[:])
    nc.finalize()  # Bacc.compile(): splits multi-sem waits (TRN2 1-wait rule)
    return nc


def _quant_host(x):
    """Exact replica of the reference quant_ste forward pass (fp32)."""
    x = np.asarray(x, np.float32)
    d = np.float32(1.0) / np.float32(128.0)
    y = np.clip(x, np.float32(-1.0) + d, np.float32(1.0) - d)
    y = y * np.float32(128.0)
    y = np.round(y)  # round-half-even, same as jnp.round
    return (y / np.float32(128.0)).astype(np.float32)


def _imajor(a):
    """[BL, IN] -> [PART, NSUPER, KCH, BS] with a[b, i] at [i%128, b//BS, i//128, b%BS]."""
    return np.ascontiguousarray(
        a.reshape(NSUPER, BS, KCH, PART).transpose(3, 0, 2, 1)
    )


def _bmajor(a):
    """[BL, OUT] -> [PART, NSUPER, G, OUT] with a[b, o] at [b%128, b//BS, (b//128)%G, o]."""
    return np.ascontiguousarray(
        a.reshape(NSUPER, G, PART, OUT).transpose(2, 0, 1, 3)
    )


_cache = {}


def kernel(**inputs):
    from concourse.bass_utils import run_bass_kernel_spmd

    P = np.asarray(inputs["P"], np.float32)
    Q = np.asarray(inputs["Q"], np.float32)
    S = np.asarray(inputs["S"], np.float32)
    W = np.asarray(inputs["weights"], np.float32)
    bias = np.asarray(inputs["bias"], np.float32)

    wq = _quant_host(W).astype(ml_dtypes.bfloat16)
    # pre-bias S on the host: spike == (E + bq - S > 0.4) == (E > S + cf);
    # fp32 host arithmetic matches what the chip would compute
    cf = (np.float32(THR) - _quant_host(bias)).astype(np.float32)
    S2 = (S + cf[None, :]).astype(np.float32)

    if "nc" not in _cache:
        _cache["nc"] = build_nc()
    nc = _cache["nc"]

    in_maps = []
    for c in range(NCORES):
        sl = slice(c * BL, (c + 1) * BL)
        in_maps.append(
            {
                "p": _imajor(P[sl]),
                "q": _imajor(Q[sl]),
                "s": _bmajor(S2[sl]),
                "w": wq,
            }
        )
    res = run_bass_kernel_spmd(nc, in_maps, list(range(NCORES)))
    _cache["last"] = res  # exec_time_ns etc. when tracing is enabled
    # device layout: o[p, si, j*OUT + o'] holds row 512*si + 128*j + p
    out = np.concatenate(
        [
            res.results[c]["o"]
            .reshape(PART, NSUPER, G, OUT)
            .transpose(1, 2, 0, 3)
            .reshape(BL, OUT)
            for c in range(NCORES)
        ],
        axis=0,
    )
    return np.ascontiguousarray(out.astype(np.float32))
